# revision 1
# baseline (speedup 1.0000x reference)
"""Trainium2 Bass kernel for CNNText: embedding gather + multi-width conv1d
+ bias/ReLU/max-pool + output matmul, data-parallel over batch on 8 NeuronCores.

Strategy per core (8 batch elements each):
  - Host: dedup words -> compact bf16 embedding table (<=32768 rows) so the
    int16-indexed transposing dma_gather can be used; pre-transpose conv
    filters/output layer on host (tiny).
  - Device: dma_gather(transpose=True) fuses the embedding gather with the
    [pos, D] -> [D, pos] transpose, landing emb_T tiles ready as matmul rhs.
    Conv(width w) = sum over kernel offsets i of emb_T shifted by i times the
    per-offset filter slice -> PSUM accumulation; ReLU(max+bias) after a
    free-dim max reduce; final [8,300]@[300,10] matmul on device.
"""
import numpy as np
import ml_dtypes
from contextlib import ExitStack

import concourse.tile as tile
from concourse import bacc, mybir
from concourse.bass_utils import run_bass_kernel_spmd

P = 128
SL = 512
D = 512
B = 64
NCORES = 8
NB = B // NCORES          # batch elems per core
LAYERNUM = 100
WIDTHS = [3, 4, 5]
NT = sum(WIDTHS)          # 12 (width, offset) filter tiles
KC = D // P               # 4 contraction chunks
VMAX = 32768              # compact table rows (max distinct words = 64*512)
DOUT = 10

BF16 = mybir.dt.bfloat16
F32 = mybir.dt.float32
I16 = mybir.dt.int16

_CACHE: dict = {}
LAST_RESULTS = None       # BassKernelResults of the most recent run (for profiling)


def _build():
    nc = bacc.Bacc("TRN2", target_bir_lowering=False, debug=False,
                   enable_asserts=True, num_devices=NCORES)

    table = nc.dram_tensor("table", [VMAX, D], BF16, kind="ExternalInput").ap()
    idx = nc.dram_tensor("idx", [P, NB * (SL // 16)], I16, kind="ExternalInput").ap()
    wts = nc.dram_tensor("wts", [P, KC * NT * LAYERNUM], BF16, kind="ExternalInput").ap()
    ol = nc.dram_tensor("ol", [LAYERNUM, 3 * DOUT], F32, kind="ExternalInput").ap()
    bias = nc.dram_tensor("bias", [LAYERNUM, 3], F32, kind="ExternalInput").ap()
    out = nc.dram_tensor("out", [NB, DOUT], F32, kind="ExternalOutput").ap()

    with tile.TileContext(nc) as tc:
        with ExitStack() as ctx:
            consts = ctx.enter_context(tc.tile_pool(name="consts", bufs=1))
            embp = ctx.enter_context(tc.tile_pool(name="emb", bufs=3))
            psump = ctx.enter_context(tc.tile_pool(name="psum", bufs=2, space="PSUM"))
            outp = ctx.enter_context(tc.tile_pool(name="outp", bufs=1))

            wt = consts.tile([P, KC, NT, LAYERNUM], BF16)
            nc.sync.dma_start(wt[:], wts.rearrange("p (c t f) -> p c t f", c=KC, t=NT))
            idx_t = consts.tile([P, NB, SL // 16], I16)
            nc.sync.dma_start(idx_t[:], idx.rearrange("p (b s) -> p b s", b=NB))
            ol_t = consts.tile([LAYERNUM, 3, DOUT], F32)
            nc.sync.dma_start(ol_t[:], ol.rearrange("p (w o) -> p w o", w=3))
            bias_t = consts.tile([LAYERNUM, 3], F32)
            nc.sync.dma_start(bias_t[:], bias)

            pooled = [outp.tile([LAYERNUM, NB], F32, tag=f"pool{wi}", name=f"pool{wi}")
                      for wi in range(3)]

            for b in range(NB):
                emb = embp.tile([P, KC, SL], BF16, tag="emb")
                nc.gpsimd.dma_gather(
                    emb[:], table[:], idx_t[:, b, :],
                    num_idxs=SL, num_idxs_reg=SL, elem_size=D,
                    transpose=True,
                )
                t0 = 0
                for wi, w in enumerate(WIDTHS):
                    ps = psump.tile([LAYERNUM, SL], F32, tag=f"ps{wi}")
                    for i in range(w):
                        for c in range(KC):
                            nc.tensor.matmul(
                                ps[:, 0:SL - i],
                                lhsT=wt[:, c, t0 + i, :],
                                rhs=emb[:, c, i:SL],
                                start=(i == 0 and c == 0),
                                stop=(i == w - 1 and c == KC - 1),
                            )
                    nc.vector.reduce_max(pooled[wi][:, b:b + 1], ps[:],
                                         axis=mybir.AxisListType.X)
                    t0 += w

            fin = psump.tile([NB, DOUT], F32, tag="fin")
            for wi in range(3):
                pr = outp.tile([LAYERNUM, NB], F32, tag=f"pr{wi}", name=f"pr{wi}")
                nc.scalar.activation(pr[:], pooled[wi][:],
                                     mybir.ActivationFunctionType.Relu,
                                     bias=bias_t[:, wi:wi + 1])
                nc.tensor.matmul(fin[:], lhsT=pr[:], rhs=ol_t[:, wi, :],
                                 start=(wi == 0), stop=(wi == 2))
            res = outp.tile([NB, DOUT], F32)
            nc.vector.tensor_copy(res[:], fin[:])
            nc.sync.dma_start(out, res[:])

    nc.compile()
    return nc


def _pack_idx(ridx):
    """[NB, SL] int16 -> [128, NB*SL/16]: position i -> partition i%16,
    col i//16, replicated over the 8 16-partition groups."""
    t16 = ridx.reshape(NB, SL // 16, 16).transpose(2, 0, 1)   # [16, NB, 32]
    return np.tile(t16, (8, 1, 1)).reshape(P, NB * (SL // 16)).copy()


def kernel(words, Embedding, outputlayer, filters_w3, bias_w3,
           filters_w4, bias_w4, filters_w5, bias_w5):
    global LAST_RESULTS
    words = np.asarray(words)
    Embedding = np.asarray(Embedding, dtype=np.float32)
    outputlayer = np.asarray(outputlayer, dtype=np.float32)
    filts = {3: np.asarray(filters_w3, dtype=np.float32),
             4: np.asarray(filters_w4, dtype=np.float32),
             5: np.asarray(filters_w5, dtype=np.float32)}
    biases = {3: np.asarray(bias_w3, dtype=np.float32),
              4: np.asarray(bias_w4, dtype=np.float32),
              5: np.asarray(bias_w5, dtype=np.float32)}

    # Dedup the vocabulary actually referenced so indices fit in int16 and
    # gather traffic shrinks (<= 32768 distinct of 50000 rows).
    uniq, inv = np.unique(words, return_inverse=True)
    table = np.zeros((VMAX, D), dtype=ml_dtypes.bfloat16)
    table[:len(uniq)] = Embedding[uniq].astype(ml_dtypes.bfloat16)
    inv = inv.reshape(B, SL).astype(np.int16)

    K_all = np.stack([filts[w].reshape(LAYERNUM, w, D)[:, i, :].T
                      for w in WIDTHS for i in range(w)])     # [12, 512, 100]
    wts = (K_all.reshape(NT, KC, P, LAYERNUM).transpose(2, 1, 0, 3)
           .reshape(P, KC * NT * LAYERNUM).astype(ml_dtypes.bfloat16))
    ol = (outputlayer.reshape(3, LAYERNUM, DOUT).transpose(1, 0, 2)
          .reshape(LAYERNUM, 3 * DOUT).copy())
    bias = np.stack([biases[w] for w in WIDTHS], axis=1).copy()

    in_maps = []
    for core in range(NCORES):
        ridx = inv[core * NB:(core + 1) * NB]
        in_maps.append({"table": table, "idx": _pack_idx(ridx),
                        "wts": wts, "ol": ol, "bias": bias})

    nc = _CACHE.get("nc")
    if nc is None:
        nc = _CACHE["nc"] = _build()

    res = run_bass_kernel_spmd(nc, in_maps, core_ids=list(range(NCORES)))
    LAST_RESULTS = res
    return np.concatenate([res.results[i]["out"] for i in range(NCORES)],
                          axis=0).astype(np.float32)
